# revision 21
# baseline (speedup 1.0000x reference)
"""Multi-head self-attention (B=4, L=2048, D=1024, H=16) on 8 trn2 cores.

Sharding: DP=4 over batch x TP=2 over heads (8 heads/core). Each core:
  QT/KT = W.T @ x.T projections (c on partitions), V natural layout,
  S^T = K Q^T per head with keys on partitions (the two 64-row head
  matmuls row-tile into the PE array and run concurrently), exp on ACT,
  causal diagonal masked by a DVE multiply with a resident tril tile,
  ctx^T = V'.T @ P^T with a ones-column producing the softmax
  denominator row.

Fused schedule: attention iterations run ib-outer/hp-inner; the QKV
projection of blocks 1-3 and the output projection of completed
i-blocks form a dense-matmul queue pumped between attention tile-steps
so the PE never idles (idle gaps trip the HAM throttle to half clock).
Output-projection PSUM reads go through DVE (not ACT - ACT is the
attention bottleneck) and out is stored f16; the host sums the TP pair
partials + b_lin in f32.
"""

import sys

if "/opt/trn_rl_repo" not in sys.path:
    sys.path.insert(0, "/opt/trn_rl_repo")

from collections import deque

import numpy as np

import concourse.bass as bass
import concourse.tile as tile
from concourse import bacc, mybir
from concourse.bass_utils import run_bass_kernel_spmd

B, L, D = 4, 2048, 1024
H, HD = 16, 64
HPC = H // 2          # heads per core (TP=2)
CPC = HPC * HD        # head-dim columns per core = 512
N_CORES = 8

F32 = mybir.dt.float32
F16 = mybir.dt.float16

NB = 4                # 512-wide i/l blocks
BW = L // NB          # 512
NT = L // 128         # 16 j/l tiles of 128
DT = D // 128         # 8 d-tiles
CT = CPC // 128       # 4 c-tiles per core

ST_BUFS = 2           # scores psum [128,2,512] = 2 banks each -> 4 banks
CX_BUFS = 2           # ctx psum [128,512] = 1 bank each -> 2 banks
DN_BUFS = 2           # dense (proj/outproj) psum -> 2 banks
PR = 4                # cross-iteration pre-roll steps


def build(tc, io):
    nc = tc.nc
    xbT = io["xbT"].rearrange("(t p) l -> t p l", p=128)      # [8,128,2048] f16
    wq = io["wq"].rearrange("(t p) c -> p t c", p=128)        # [128,8,512] view
    wk = io["wk"].rearrange("(t p) c -> p t c", p=128)
    wv = io["wv"].rearrange("(t p) c -> p t c", p=128)
    wo = io["wo"].rearrange("(t p) e -> p t e", p=128)        # [128,4,1024]
    bq = io["bq"].rearrange("(t p) -> p t", p=128)            # [128,4] f32
    bk = io["bk"].rearrange("(t p) -> p t", p=128)
    out = io["out"].rearrange("(t p) e -> t p e", p=128)      # [16,128,1024] f16

    singles = tc.alloc_tile_pool(name="singles", bufs=1)
    xpool = tc.alloc_tile_pool(name="xpool", bufs=32)
    work = tc.alloc_tile_pool(name="work", bufs=4)
    psum = tc.alloc_tile_pool(name="psum", bufs=4, space="PSUM")

    # --- resident tensors; first projection needs only (xt dt, wq dt) pairs,
    # so interleave those DMAs per d-tile to minimize time-to-first-matmul.
    wq_sb = singles.tile([128, DT, CPC], F16)
    wk_sb = singles.tile([128, DT, CPC], F16)
    wv_sb = singles.tile([128, DT, CPC], F16)
    wo_sb = singles.tile([128, CT, D], F16)
    bq_sb = singles.tile([128, CT], F32)
    bk_sb = singles.tile([128, CT], F32)
    bv_row = singles.tile([1, CPC], F32)
    bv128 = singles.tile([128, CPC], F32)
    mask_sb = singles.tile([128, 128], F16)   # tril: m[j,i]=1 iff j<=i
    nc.sync.dma_start(out=bq_sb, in_=bq)
    nc.sync.dma_start(out=bk_sb, in_=bk)
    nc.sync.dma_start(out=bv_row, in_=io["bvr"].rearrange("(o c) -> o c", o=1))
    nc.sync.dma_start(out=mask_sb, in_=io["msk"])
    # first compute pieces are (q ct0, k ct0, v) of block 0: interleave their
    # operands per d-tile so each matmul can start as its data lands.
    xts = {}
    xts[0] = [xpool.tile([128, BW], F16, tag="xbt", name=f"xt0_{dt}")
              for dt in range(DT)]
    for dt in range(DT):
        nc.sync.dma_start(out=xts[0][dt], in_=xbT[dt, :, 0:BW])
        nc.sync.dma_start(out=wq_sb[:, dt, 0:128], in_=wq[:, dt, 0:128])
        nc.sync.dma_start(out=wk_sb[:, dt, 0:128], in_=wk[:, dt, 0:128])
        nc.sync.dma_start(out=wv_sb[:, dt, :], in_=wv[:, dt, :])
    nc.gpsimd.partition_broadcast(bv128, bv_row)
    bv128v = bv128.rearrange("p (h d) -> p h d", h=HPC)
    for dt in range(DT):
        nc.sync.dma_start(out=wq_sb[:, dt, 128:], in_=wq[:, dt, 128:])
        nc.sync.dma_start(out=wk_sb[:, dt, 128:], in_=wk[:, dt, 128:])
    for blk in range(1, NB):
        xts[blk] = [xpool.tile([128, BW], F16, tag="xbt", name=f"xt{blk}_{dt}")
                    for dt in range(DT)]
        for dt in range(DT):
            nc.sync.dma_start(out=xts[blk][dt],
                              in_=xbT[dt, :, blk * BW : (blk + 1) * BW])
    nc.sync.dma_start(out=wo_sb, in_=wo)

    qt_sb = singles.tile([128, CT, L], F16)    # Q^T  [c, i]
    kt_sb = singles.tile([128, CT, L], F16)    # K^T  [c, j]
    vp_sb = singles.tile([128, NT, HPC, HD + 1], F16)   # V' [j, (h, d'|1)]
    cxt_sb = singles.tile([128, CT, L], F16)   # normalized ctx^T [d', i]
    nc.gpsimd.memset(vp_sb[:, :, :, HD : HD + 1], 1.0)

    # ---- dense work: projection blocks + output projection, as generators
    # yielding every ~2 matmuls (one PE "fill unit"). Elementwise finalizers
    # run on GPSIMD (the only lightly-loaded engine) so the dense psum banks
    # are freed promptly - DVE's deep in-order queue would hold them.
    dummy = singles.tile([128, BW], F16)
    # single psum tile shared by all warm-up matmuls: no ring rotation, so
    # they serialize on WAW only and never wait on a real chunk's readers.
    warm_ps = psum.tile([128, BW], F32, tag="dn", bufs=DN_BUFS, name="warm")

    def warm_mm():
        nc.tensor.matmul(warm_ps, lhsT=dummy[:, 0:128], rhs=dummy,
                         start=True, stop=True)

    def gen_qk(blk, ct4, warm=False):
        xt = xts[blk]
        for w_sb, b_sb, t_sb in ((wq_sb, bq_sb, qt_sb), (wk_sb, bk_sb, kt_sb)):
            ps = psum.tile([128, BW], F32, tag="dn", bufs=DN_BUFS,
                           name=f"pp{blk}{ct4}")
            for dt in range(DT):
                nc.tensor.matmul(
                    ps, lhsT=w_sb[:, dt, ct4 * 128 : (ct4 + 1) * 128],
                    rhs=xt[dt], start=(dt == 0), stop=(dt == DT - 1))
                if warm:
                    # DMA-paced first tile: keep the PE array warm (HAM
                    # drops to half clock after idle) with a throwaway
                    # matmul between the gated real ones.
                    warm_mm()
                if dt % 2 == 1 and dt < DT - 1:
                    yield
            warm = False
            # bias-add on ACT (idle during proj phases; GPSIMD has no
            # PSUM port, and DVE's deep queue would hold the bank).
            nc.scalar.activation(
                out=t_sb[:, ct4, blk * BW : (blk + 1) * BW], in_=ps,
                func=mybir.ActivationFunctionType.Identity,
                bias=b_sb[:, ct4 : ct4 + 1])
            yield

    def gen_v(blk):
        xt = xts[blk]
        for j in range(4):
            lt = 4 * blk + j
            ps = psum.tile([128, CPC], F32, tag="dn", bufs=DN_BUFS,
                           name=f"psv{lt}")
            for dt in range(DT):
                nc.tensor.matmul(
                    ps, lhsT=xt[dt][:, j * 128 : (j + 1) * 128],
                    rhs=wv_sb[:, dt, :], start=(dt == 0), stop=(dt == DT - 1))
                if dt % 2 == 1 and dt < DT - 1:
                    yield
            # fold bv into V': softmax-averaging ones gives +bv exactly.
            nc.vector.tensor_tensor(
                out=vp_sb[:, lt, :, 0:HD],
                in0=ps.rearrange("p (h d) -> p h d", h=HPC),
                in1=bv128v, op=mybir.AluOpType.add)
            yield

    def outproj_gen(ib):
        # Per i-tile: both halves' ct0-2 accumulate first, the ct3
        # contribution (gated on the slowest head-pair's normalization)
        # lands last so the wait overlaps the ct0-2 matmuls. The psum
        # read-out runs on ACT for the last block (ACT is idle once the
        # exps are done; DVE is still busy with the final normalization).
        for j in range(4):
            it = 4 * ib + j
            pss = []
            for eb in range(2):
                ps = psum.tile([128, 512], F32, tag="dn", bufs=DN_BUFS,
                               name=f"po{it}_{eb}")
                for ct in range(CT - 1):
                    nc.tensor.matmul(
                        ps, lhsT=cxt_sb[:, ct, it * 128 : (it + 1) * 128],
                        rhs=wo_sb[:, ct, eb * 512 : (eb + 1) * 512],
                        start=(ct == 0), stop=False)
                pss.append(ps)
                yield
            for eb in range(2):
                nc.tensor.matmul(
                    pss[eb], lhsT=cxt_sb[:, CT - 1, it * 128 : (it + 1) * 128],
                    rhs=wo_sb[:, CT - 1, eb * 512 : (eb + 1) * 512],
                    start=False, stop=True)
                o_sb = work.tile([128, 512], F16, tag="osb", bufs=4,
                                 name=f"os{it}_{eb}")
                if ib == NB - 1:
                    nc.scalar.copy(out=o_sb, in_=pss[eb])
                else:
                    nc.vector.tensor_copy(out=o_sb, in_=pss[eb])
                nc.sync.dma_start(out=out[it][:, eb * 512 : (eb + 1) * 512],
                                  in_=o_sb)
                yield

    # ---- attention iteration (hp, ib): ntj tile-steps + normalization.
    # Steps taken while pre_mode is set (pre-rolled into the predecessor's
    # tail) emit scores+exp only; their ctx matmuls are deferred until the
    # iteration's own slot. With ctx buffers at 2 (one iteration pair in
    # flight), an early-emitted ctx would wait - at the head of the in-order
    # PE queue - on a buffer freed only by PE work queued behind it.
    pre_mode = [False]

    def attn_gen(hp, ib):
        ctx_ps = [psum.tile([128, BW], F32, tag="cx", bufs=CX_BUFS,
                            name=f"cx{hp}_{ib}_{hh}") for hh in range(2)]
        ntj = 4 * ib + 4

        def scores(tj):
            k = tj - 4 * ib
            off = 128 * k if k >= 0 else 0
            st = psum.tile([128, 2, BW], F32, tag="st", bufs=ST_BUFS,
                           name=f"st{hp}_{ib}_{tj}")
            pt = work.tile([128, 2, BW], F16, tag="pt", bufs=10,
                           name=f"pt{hp}_{ib}_{tj}")
            for hh in range(2):
                nc.tensor.matmul(
                    st[:, hh, off:], skip_group_check=True, start=True, stop=True,
                    lhsT=kt_sb[64 * hh : 64 * hh + 64, hp,
                               tj * 128 : (tj + 1) * 128],
                    rhs=qt_sb[64 * hh : 64 * hh + 64, hp,
                              ib * BW + off : (ib + 1) * BW])
            nc.scalar.activation(
                out=pt[:, :, off:], in_=st[:, :, off:],
                func=mybir.ActivationFunctionType.Exp, scale=0.125)
            if k >= 0:
                # zero j>i in the 128-wide diagonal sub-block, in place
                for hh in range(2):
                    nc.vector.tensor_tensor(
                        out=pt[:, hh, off : off + 128],
                        in0=pt[:, hh, off : off + 128],
                        in1=mask_sb, op=mybir.AluOpType.mult)
            return tj, off, pt

        def ctx(tj, off, pt):
            for hh in range(2):
                nc.tensor.matmul(
                    ctx_ps[hh][0 : HD + 1, off:], skip_group_check=True,
                    lhsT=vp_sb[:, tj, 2 * hp + hh, :],
                    rhs=pt[:, hh, off:], start=(tj == 0), stop=(tj == ntj - 1))

        # software pipeline: scores(tj+1) issued before ctx(tj) so the PE
        # queue alternates [st pair | ctx pair] with 1-tile lookahead.
        pend = []
        for tj in range(ntj):
            pend.append(scores(tj))
            if not pre_mode[0] and len(pend) > 2:
                ctx(*pend.pop(0))
            yield
        for p in pend:
            ctx(*p)
        rdens = []
        for hh in range(2):
            # reciprocal_approx_fast misreads non-zero partition bases;
            # stage the denominator row at partition 0 first.
            den = work.tile([1, BW], F32, tag="den", name=f"dn{hp}{ib}{hh}")
            nc.vector.tensor_copy(out=den, in_=ctx_ps[hh][HD : HD + 1, :])
            rden = work.tile([1, BW], F32, tag="rden", name=f"rd{hp}{ib}{hh}")
            nc.vector.reciprocal_approx_fast(out=rden, in_=den)
            rdens.append(rden)
        rbs = []
        for hh in range(2):
            rb = work.tile([64, BW], F32, tag="rb", name=f"rb{hp}{ib}{hh}")
            nc.gpsimd.partition_broadcast(rb, rdens[hh])
            rbs.append(rb)
        for hh in range(2):
            nc.vector.tensor_tensor(
                out=cxt_sb[64 * hh : 64 * hh + 64, hp, ib * BW : (ib + 1) * BW],
                in0=ctx_ps[hh][0:HD, :], in1=rbs[hh],
                op=mybir.AluOpType.mult)

    # ---- fused master schedule --------------------------------------------
    # Dense inventory in deadline order: (qk blk ct) is needed by attention
    # iteration (ib=blk, hp=ct); (v blk) by (ib=blk, hp=0)'s first ctx.
    # Output-projection units have no deadline, so they are HELD BACK and
    # released for the late, exp-paced stretch where projection inventory
    # has run out (else the PE idles there and HAM halves the clock).
    piece_of = {}
    dq = deque()
    for blk in range(NB):
        for key, g in ((("qk", blk, 0), gen_qk(blk, 0, warm=(blk == 0))),
                       (("v", blk), gen_v(blk)),
                       (("qk", blk, 1), gen_qk(blk, 1)),
                       (("qk", blk, 2), gen_qk(blk, 2)),
                       (("qk", blk, 3), gen_qk(blk, 3))):
            piece_of[key] = g
            dq.append(g)
    done = set()

    state = {"units": 4 * 48.0 + 4 * 16.0, "steps": 160.0, "acc": 0.0}
    op_hold = []

    def _advance(g):
        try:
            next(g)
            state["units"] -= 1
            return True
        except StopIteration:
            done.add(id(g))
            return False

    def pump_step():
        state["steps"] -= 1
        if state["steps"] <= 85 and op_hold:
            dq.extend(op_hold)
            del op_hold[:]
        state["acc"] = min(state["acc"] + state["units"] / max(state["steps"], 1.0), 4.0)
        while state["acc"] >= 1.0 and dq:
            if _advance(dq[0]):
                state["acc"] -= 1.0
            else:
                dq.popleft()

    def ensure(key):
        g = piece_of.get(key)
        if g is None or id(g) in done:
            return
        while dq and dq[0] is not g:
            if not _advance(dq[0]):
                dq.popleft()
        while _advance(g):
            pass
        if dq and dq[0] is g:
            dq.popleft()

    def marker(ib, hp):
        return ("v", ib) if hp == 0 else ("qk", ib, hp)

    # PE warm-up: throwaway matmuls bridge the initial DMA latency so HAM
    # reaches full clock before the first real matmul; then the minimum
    # pieces for attention (ib0, hp0): q/k c-tile 0 and V of block 0.
    nc.gpsimd.memset(dummy, 0.0)
    for _ in range(28):
        warm_mm()
    ensure(("qk", 0, 0))
    ensure(("v", 0))

    # Attention ib-outer / hp-inner, with the successor's first steps
    # pre-rolled into each iteration's tail and dense units pumped at
    # every tile-step boundary.
    gens = [(attn_gen(hp, ib), 4 * ib + 4, hp, ib)
            for ib in range(NB) for hp in range(CT)]
    _S = object()
    carry = 0
    for idx, (g, ntj, hp, ib) in enumerate(gens):
        ensure(marker(ib, hp))
        if idx + 1 < len(gens):
            _, nxt_ntj, hp2, ib2 = gens[idx + 1]
            nxt = gens[idx + 1][0]
            ensure(marker(ib2, hp2))     # pre-rolled scores need its q/k
        else:
            nxt, nxt_ntj = None, 0
        pre = 0
        for i in range(carry, ntj):
            next(g, _S)                      # one tile-step of this iteration
            pump_step()
            if nxt is not None and i >= ntj - PR and pre < nxt_ntj:
                pre_mode[0] = True           # successor step: scores+exp only
                next(nxt, _S)
                pre_mode[0] = False
                pump_step()
                pre += 1
        next(g, _S)                          # tail: last ctx + normalization
        carry = pre
        if hp == 3:
            op = outproj_gen(ib)
            if state["steps"] <= 85:
                dq.append(op)
            else:
                op_hold.append(op)
    while dq:
        if not _advance(dq[0]):
            dq.popleft()

    for p in (psum, work, xpool, singles):
        p.release()


_CACHE = {}


def _compiled():
    if "nc" in _CACHE:
        return _CACHE["nc"]
    nc = bacc.Bacc("TRN2", target_bir_lowering=False, debug=False)
    io = {
        "xbT": nc.dram_tensor("xbT", [D, L], F16, kind="ExternalInput").ap(),
        "wq": nc.dram_tensor("wq", [D, CPC], F16, kind="ExternalInput").ap(),
        "wk": nc.dram_tensor("wk", [D, CPC], F16, kind="ExternalInput").ap(),
        "wv": nc.dram_tensor("wv", [D, CPC], F16, kind="ExternalInput").ap(),
        "wo": nc.dram_tensor("wo", [CPC, D], F16, kind="ExternalInput").ap(),
        "bq": nc.dram_tensor("bq", [CPC], F32, kind="ExternalInput").ap(),
        "bk": nc.dram_tensor("bk", [CPC], F32, kind="ExternalInput").ap(),
        "bvr": nc.dram_tensor("bvr", [CPC], F32, kind="ExternalInput").ap(),
        "msk": nc.dram_tensor("msk", [128, 128], F16, kind="ExternalInput").ap(),
        "out": nc.dram_tensor("out", [L, D], F16, kind="ExternalOutput").ap(),
    }
    with tile.TileContext(nc) as tc:
        build(tc, io)
    nc.compile()
    _CACHE["nc"] = nc
    return nc


def make_in_maps(x, W_qkv, b_qkv, W_lin):
    f16 = mybir.dt.np(F16)
    msk = np.triu(np.ones((128, 128), dtype=f16))
    in_maps = []
    for c in range(N_CORES):
        b, g = divmod(c, 2)
        cs = slice(CPC * g, CPC * (g + 1))
        in_maps.append({
            "xbT": np.ascontiguousarray(x[b].T).astype(f16),
            "wq": W_qkv[:, cs].astype(f16),
            "wk": W_qkv[:, D + CPC * g : D + CPC * (g + 1)].astype(f16),
            "wv": W_qkv[:, 2 * D + CPC * g : 2 * D + CPC * (g + 1)].astype(f16),
            "wo": W_lin[cs, :].astype(f16),
            "bq": b_qkv[cs].astype(np.float32),
            "bk": b_qkv[D + CPC * g : D + CPC * (g + 1)].astype(np.float32),
            "bvr": b_qkv[2 * D + CPC * g : 2 * D + CPC * (g + 1)].astype(np.float32),
            "msk": msk,
        })
    return in_maps


def kernel(x, W_qkv, b_qkv, W_lin, b_lin, _trace=False):
    nc = _compiled()
    in_maps = make_in_maps(x, W_qkv, b_qkv, W_lin)
    res = run_bass_kernel_spmd(nc, in_maps, core_ids=list(range(N_CORES)),
                               trace=_trace)
    parts = [r["out"] for r in res.results]
    out = np.empty((B, L, D), dtype=np.float32)
    for b in range(B):
        out[b] = (parts[2 * b].astype(np.float32)
                  + parts[2 * b + 1].astype(np.float32)
                  + b_lin.astype(np.float32))
    if _trace:
        return out, res
    return out
